# revision 8
# baseline (speedup 1.0000x reference)
"""Trainium2 Bass kernel for LocalDenseConv2D + BatchNorm + PReLU.

Problem (hardcoded shapes):
  x:      (8, 64, 64, 256)  f32   (B, IN_C, L, T)
  weight: (576, 64, 64)     f32   (K = IN_C*9, OUT_C, OUT_L)  k = ci*9 + di*3 + dj
  bias:   (64, 64)          f32   (OUT_C, OUT_L)
  gamma, beta: (64,)        f32
  alpha:  (1,)              f32   (0 <= alpha <= 1 assumed for the DVE max trick)
  out:    (8, 64, 64, 256)  f32

Sharding: out_l across 8 cores (8 rows each), all batches per core.
Inputs are converted to bf16 on the host (PE streams 1 cycle/row and input
DMA halves).  Conv uses the 64x64 PE quadrant mode: row groups = batch
half (bh) on SBUF partitions, col groups = b4 pair (nt) on PSUM
partitions; four matmuls run concurrently (measured ~2.8 rows/cycle).
147456 psum rows per core, 288 matmuls, K=64 over in-channels per tap.

BatchNorm: per-partition bn_stats/bn_aggr -> (sum, sumsq); the two
partition halves are pre-merged via a tiny DRAM round-trip (hidden in the
collective's straggler shadow); AllReduce(add) of [128,2] gives global
(sum, sumsq) on every partition directly.  BN-apply + PReLU: scalar-engine
Prelu for lps 0-4, DVE affine+max(alpha*y, y) (bf16 2x mode) for lps 5-7.
The bias pass also uses Prelu (alpha=1) and a dummy Sqrt is issued early
so only one activation table set is ever loaded.  Output stored bf16,
upcast on host.
"""
import os
import sys
import numpy as np

if '/opt/trn_rl_repo' not in sys.path:
    sys.path.insert(0, '/opt/trn_rl_repo')

import concourse.bass as bass
import concourse.bacc as bacc
import concourse.mybir as mybir
import concourse.tile as tile
from concourse.bass_utils import run_bass_kernel_spmd

import ml_dtypes

BF16_NP = ml_dtypes.bfloat16
F32 = mybir.dt.float32
BF16 = mybir.dt.bfloat16
AF = mybir.ActivationFunctionType
ALU = mybir.AluOpType

B, IN_C, L, T = 8, 64, 64, 256
OUT_C, OUT_L = 64, 64
NCORES = 8
L_LOC = L // NCORES          # 8 out_l rows per core
SLAB = L_LOC + 2             # 10 x-rows incl. halo
TP = T + 2                   # padded t
EPS = 1e-5
N_PART = L_LOC * 2 * T * 2   # elems per channel-half partition = 8192
N_GLOBAL = B * L * T         # 131072
N_ACT_LP = 4                 # final-pass lps on the scalar engine; rest DVE

_cache = {}


def _build():
    nc = bacc.Bacc("TRN2", target_bir_lowering=False, debug=False,
                   num_devices=NCORES)
    bt_d = nc.dram_tensor("bt", [128, L_LOC], F32, kind="ExternalInput")
    g_d = nc.dram_tensor("g", [128, 1], F32, kind="ExternalInput")
    e_d = nc.dram_tensor("e", [128, 1], F32, kind="ExternalInput")
    a_d = nc.dram_tensor("a", [128, 1], F32, kind="ExternalInput")
    yo = nc.dram_tensor("yo", [128, L_LOC, 1024], BF16, kind="ExternalOutput")
    xa_d = nc.dram_tensor("xa", [128, 4, SLAB, TP], BF16,
                          kind="ExternalInput")
    wA_d = nc.dram_tensor("wA", [128, L_LOC, 9, OUT_C], BF16,
                          kind="ExternalInput")
    cc_in = nc.dram_tensor("cc_in", [128, 2], F32)
    cc_out = nc.dram_tensor("cc_out", [NCORES * 128, 2], F32,
                            addr_space="Shared")

    with tile.TileContext(nc) as tc:
        with (
            tc.tile_pool(name="const", bufs=1) as cpool,
            tc.tile_pool(name="xp", bufs=1) as xpool,
            tc.tile_pool(name="op", bufs=1) as opool,
            tc.tile_pool(name="fp", bufs=3) as fpool,
            tc.tile_pool(name="fd", bufs=3) as dpool,
            tc.tile_pool(name="ps", bufs=3, space="PSUM") as ppool,
        ):
            bt = cpool.tile([128, L_LOC], F32)
            gt = cpool.tile([128, 1], F32)
            et = cpool.tile([128, 1], F32)
            att = cpool.tile([128, 1], F32)
            onet = cpool.tile([128, 1], F32)
            epst = cpool.tile([128, 1], F32)
            scr = cpool.tile([128, 1], F32)
            nc.vector.memset(onet[:], 1.0)
            nc.vector.memset(epst[:], EPS)
            # dummy sqrt: forces the sqrt+prelu act table load off the
            # critical path (overlaps the input DMA)
            nc.scalar.activation(scr[:], epst[:], AF.Sqrt, bias=epst[:])

            wA = cpool.tile([128, L_LOC, 9, OUT_C], BF16)
            xa = xpool.tile([128, 4, SLAB, TP], BF16)
            ott = opool.tile([128, L_LOC, 1024], BF16)
            stats = cpool.tile([128, L_LOC, 2, 6], F32)

            # weights for lp 0-1 first so conv can start early; x rows
            # alternate sync/scalar; consts + late weights on gpsimd
            nc.scalar.dma_start(wA[:, 0:2], wA_d.ap()[:, 0:2])
            nc.gpsimd.dma_start(bt[:], bt_d.ap())
            nc.gpsimd.dma_start(gt[:], g_d.ap())
            nc.gpsimd.dma_start(et[:], e_d.ap())
            nc.gpsimd.dma_start(att[:], a_d.ap())
            nc.gpsimd.dma_start(wA[:, 2:L_LOC], wA_d.ap()[:, 2:L_LOC])
            for s in range(SLAB):
                q = nc.sync if s % 2 == 0 else nc.scalar
                q.dma_start(xa[:, :, s, :], xa_d.ap()[:, :, s, :])

            # ---- conv: 4 PE quadrants (row=bh, col=nt) ----
            for lp in range(L_LOC):
                pt = ppool.tile([128, 1024], F32, tag="pt")
                for combo in range(9):
                    di, dj = combo // 3, combo % 3
                    first = combo == 0
                    last = combo == 8
                    for bh in range(2):
                        for nt in range(2):
                            lhsT = wA[bh * 64:(bh + 1) * 64, lp, combo, :]
                            rhs = xa[bh * 64:(bh + 1) * 64,
                                     2 * nt:2 * nt + 2, lp + di, dj:dj + T]
                            nc.tensor.matmul(
                                pt[nt * 64:(nt + 1) * 64,
                                   bh * 512:(bh + 1) * 512],
                                lhsT, rhs, start=first, stop=last)

                # bias + copy to SBUF (Prelu with alpha=1 == identity)
                nc.scalar.activation(ott[:, lp, :], pt[:, :], AF.Prelu,
                                     bias=bt[:, lp:lp + 1], alpha=onet[:])
                for h in range(2):
                    nc.vector.bn_stats(stats[:, lp, h, :],
                                       pt[:, h * 512:(h + 1) * 512])

            # ---- local stats -> per-half (sum, sumsq) -> AllGather ----
            loc = cpool.tile([128, 2], F32)
            nc.vector.bn_aggr(loc[:], stats[:].rearrange("p a b c -> p (a b c)"))
            msq = cpool.tile([128, 1], F32)
            ccs = cpool.tile([128, 2], F32)
            nc.vector.tensor_tensor(msq[:], loc[:, 0:1], loc[:, 0:1], ALU.mult)
            nc.vector.tensor_tensor(loc[:, 1:2], loc[:, 1:2], msq[:], ALU.add)
            nc.vector.tensor_scalar_mul(ccs[:], loc[:], float(N_PART))
            nc.sync.dma_start(cc_in.ap(), ccs[:])
            nc.gpsimd.collective_compute(
                "AllGather", ALU.bypass,
                replica_groups=[list(range(NCORES))],
                ins=[cc_in[:]], outs=[cc_out[:]])

            # ---- gather all 16 (rank, half) partials per channel ----
            gm = cpool.tile([128, 16, 2], F32)
            srcv = cc_out.ap().rearrange("(h r q c) s -> c h (r q) s",
                                         h=2, r=NCORES // 2, q=2, c=64)
            nc.sync.dma_start(gm[0:64, 0:8], srcv[:, 0])
            nc.scalar.dma_start(gm[0:64, 8:16], srcv[:, 1])
            nc.sync.dma_start(gm[64:128, 0:8], srcv[:, 0])
            nc.scalar.dma_start(gm[64:128, 8:16], srcv[:, 1])
            tot = cpool.tile([128, 2], F32)
            nc.vector.tensor_reduce(tot[:], gm[:].rearrange("p a s -> p s a"),
                                    axis=mybir.AxisListType.X, op=ALU.add)
            mv = cpool.tile([128, 2], F32)       # (mean, E[x^2])
            nc.vector.tensor_scalar_mul(mv[:], tot[:], 1.0 / N_GLOBAL)
            msq2 = cpool.tile([128, 1], F32)
            var = cpool.tile([128, 1], F32)
            nc.vector.tensor_tensor(msq2[:], mv[:, 0:1], mv[:, 0:1], ALU.mult)
            nc.vector.tensor_tensor(var[:], mv[:, 1:2], msq2[:], ALU.subtract)
            std = cpool.tile([128, 1], F32)
            rstd = cpool.tile([128, 1], F32)
            sca = cpool.tile([128, 1], F32)
            shi = cpool.tile([128, 1], F32)
            nc.scalar.activation(std[:], var[:], AF.Sqrt, bias=epst[:])
            nc.vector.reciprocal(rstd[:], std[:])
            nc.vector.tensor_tensor(sca[:], gt[:], rstd[:], ALU.mult)
            nc.vector.tensor_tensor(shi[:], mv[:, 0:1], sca[:], ALU.mult)
            nc.vector.tensor_tensor(shi[:], et[:], shi[:], ALU.subtract)

            # ---- fused BN-apply + PReLU + store (bf16) ----
            out_q = [nc.sync, nc.gpsimd]
            for lp in range(L_LOC):
                if lp < N_ACT_LP:
                    fo = fpool.tile([128, 1024], BF16, tag="fo")
                    nc.scalar.activation(fo[:], ott[:, lp, :], AF.Prelu,
                                         bias=shi[:], scale=sca[:],
                                         alpha=att[:])
                else:
                    # y = x*sca + shi; out = max(y, alpha*y)  (alpha <= 1)
                    fo = dpool.tile([128, 1024], BF16, tag="fo_d")
                    y = dpool.tile([128, 1024], BF16, tag="y")
                    ay = dpool.tile([128, 1024], BF16, tag="ay")
                    nc.vector.tensor_scalar(y[:], ott[:, lp, :], sca[:],
                                            shi[:], ALU.mult, ALU.add)
                    nc.vector.tensor_scalar(ay[:], y[:], att[:], None,
                                            ALU.mult)
                    nc.vector.tensor_tensor(fo[:], y[:], ay[:], ALU.max)
                out_q[lp % 2].dma_start(yo.ap()[:, lp, :], fo[:])
    nc.compile()
    return nc


def _prep(x, weight, bias, gamma, beta, alpha):
    """Build per-core input maps (host-side shard + relayout, bf16)."""
    xpad = np.zeros((B, IN_C, L + 2, TP), np.float32)
    xpad[:, :, 1:L + 1, 1:T + 1] = x
    xpad = xpad.astype(BF16_NP)
    wl = weight.reshape(IN_C, 9, OUT_C, OUT_L).astype(BF16_NP)

    in_maps = []
    for r in range(NCORES):
        l0 = r * L_LOC
        slab = xpad[:, :, l0:l0 + SLAB, :]          # (B, C, 10, 258)
        xa = np.ascontiguousarray(
            slab.reshape(2, 4, IN_C, SLAB, TP).transpose(0, 2, 1, 3, 4)
            .reshape(128, 4, SLAB, TP))
        wv = wl[:, :, :, l0:l0 + L_LOC]             # (ci, combo, c, lp)
        wA = np.ascontiguousarray(
            np.broadcast_to(
                wv.transpose(0, 3, 1, 2)[None],     # (ci, lp, combo, c)
                (2, IN_C, L_LOC, 9, OUT_C))
            .reshape(128, L_LOC, 9, OUT_C))
        in_maps.append({
            "xa": xa, "wA": wA,
            "bt": np.ascontiguousarray(
                np.tile(bias[:, l0:l0 + L_LOC], (2, 1))).astype(np.float32),
            "g": np.tile(gamma.reshape(-1, 1), (2, 1)).astype(np.float32),
            "e": np.tile(beta.reshape(-1, 1), (2, 1)).astype(np.float32),
            "a": np.full((128, 1), float(alpha[0]), np.float32),
        })
    return in_maps


def kernel(x, weight, bias, gamma, beta, alpha, trace=False,
           trace_cores=None):
    x = np.asarray(x, np.float32)
    weight = np.asarray(weight, np.float32)
    bias = np.asarray(bias, np.float32)
    gamma = np.asarray(gamma, np.float32)
    beta = np.asarray(beta, np.float32)
    alpha = np.asarray(alpha, np.float32)

    if "nc" not in _cache:
        _cache["nc"] = _build()
    nc = _cache["nc"]
    in_maps = _prep(x, weight, bias, gamma, beta, alpha)
    kwargs = {}
    if trace_cores is not None:
        kwargs["trace_cores"] = trace_cores
    res = run_bass_kernel_spmd(nc, in_maps, list(range(NCORES)), trace=trace,
                               **kwargs)
    kernel._last = res

    out = np.empty((B, OUT_C, L, T), np.float32)
    for r in range(NCORES):
        yr = np.asarray(res.results[r]["yo"]).astype(np.float32)
        l0 = r * L_LOC
        # partition (nt, c); free (bh, b4r, t); b = bh*4 + nt*2 + b4r
        a6 = yr.reshape(2, 64, L_LOC, 2, 2, 256)
        blk = a6.transpose(3, 0, 4, 1, 2, 5)
        out[:, :, l0:l0 + L_LOC, :] = blk.reshape(B, OUT_C, L_LOC, T)
    return out
